# revision 19
# baseline (speedup 1.0000x reference)
"""LocalPoolPointnet on 8 Trainium2 cores — v3.

Structure per core (points sorted by bin, contiguous bin range per core):
  - net stored fp32 in SBUF; bf16 "hi" parts are free stride-2 bitcast views.
  - scatter_mean: PE transposes + bf16 hi/lo one-hot matmuls into a 512-wide
    window per 512-pt chunk, accumulated into an SBUF `sums` buffer; means are
    formed segment-wise (sums * 1/cnt with a DMA-broadcast reciprocal row).
  - pooled-path matmuls are computed per-BIN (pb = gelu(mean)@W0b,
    psb = mean@Wsb), stored interleaved, and fetched per-point with a single
    GPSIMD ap_gather (d=2, f32); the gathered values are folded into the
    existing PSUM-evac scalar_tensor_tensor ops for free.
  - per-point matmuls: w0a/w1 in fp32 (4 cyc/row, no operand prep);
    wsa/wc as bf16 hi/lo 3-pass (hi is a free bitcast view of net).
"""

import sys
import numpy as np

# ---------------------------------------------------------------- constants
B = 2
NP_ = 100_000
HID = 128
D2 = 256
NBLK = 5
RES = 64
R = 20_005          # max_coord_num in the reference
BIG = RES ** 3 + 1
NCORES = 8
CORES_PER_BATCH = NCORES // B

NPTS = 25_600       # padded points per core (= 50 * 512 = 200 * 128)
NCHUNK = NPTS // 512            # 50
NTILES = NPTS // 128            # 200
WIN = 512                       # scatter bin window per 512-pt chunk
NBINS = 5_888                   # sums capacity per core (46*128, incl. window slack)
NRANK = 46                      # mean-table ranks (bin-major wrap)
NROWS = NRANK * 128             # 5888 rows in the DRAM mean table
NBIAS = 13                      # b_pos(2) b0(5) b1(5) b_c(1)
FW = NPTS * 4 // 128            # 800

F32 = np.float32


def _bf16(x):
    xi = np.ascontiguousarray(x, F32).view(np.uint32)
    return ((xi + 0x8000) & 0xFFFF0000).view(F32)


# ================================================================ host prep
def point_meta(p, sparse_coords, res):
    """Integer routing metadata, bit-identical to the reference's indexing."""
    p = np.asarray(p, F32)
    sc = np.asarray(sparse_coords)
    coord = np.clip(p + F32(0.5), F32(1e-6), F32(1.0 - 1e-6)) * F32(res)
    cl = coord.astype(np.int32)
    lin = (cl[..., 0] * res + cl[..., 1]) * res + cl[..., 2]      # [B, NP]

    slin = (sc[:, 1] * res + sc[:, 2]) * res + sc[:, 3]
    index = np.empty((B, NP_), np.int64)
    for b in range(B):
        coords_b = np.sort(np.where(sc[:, 0] == b, slin, BIG))
        index[b] = np.searchsorted(coords_b, lin[b], side="left")
    counts = np.bincount(sc[:, 0], minlength=B)
    return index, counts


def shard(p, index):
    """Split each batch's points into CORES_PER_BATCH contiguous-bin shards."""
    shards = []
    for b in range(B):
        idx = index[b]
        order = np.argsort(idx, kind="stable")
        sidx = idx[order]
        binc = np.bincount(idx, minlength=R)
        csum = np.cumsum(binc)
        prev_hi = 0
        for c in range(CORES_PER_BATCH):
            if c < CORES_PER_BATCH - 1:
                target = (c + 1) * NP_ // CORES_PER_BATCH
                hi = int(np.searchsorted(csum, target))
                if hi > 0 and target - csum[hi - 1] < csum[hi] - target:
                    hi -= 1
                hi += 1          # shard owns bins [lo, hi)
            else:
                hi = R
            lo = prev_hi
            prev_hi = hi
            sel = slice(int(np.searchsorted(sidx, lo)), int(np.searchsorted(sidx, hi)))
            pts = p[b][order[sel]]                     # [n, 3] sorted by bin
            rel = (sidx[sel] - lo).astype(np.int64)    # sorted rel bins
            assert pts.shape[0] <= NPTS, f"core shard too big: {pts.shape[0]}"
            nb = hi - lo
            assert nb + WIN <= NBINS, f"bin shard too big: {nb}"
            shards.append(dict(batch=b, lo=lo, hi=hi, pts=pts, rel=rel, nb=nb))
    return shards


def _wrap_rep(ix):
    """idx layout for gpsimd gathers: unwrapped[i] = w[i%16, i//16], replicated
    into every 16-partition group (each Q7 core reads its own copy)."""
    w = np.zeros((128, ix.shape[0] // 16), np.int16)
    pat = ix.reshape(-1, 16).T
    for g in range(8):
        w[g * 16:(g + 1) * 16] = pat
    return w


def core_inputs(sh):
    """Per-core padded arrays for the device kernel."""
    n = sh["pts"].shape[0]
    pts = np.full((NPTS, 3), 0.25, F32)
    pts[:n] = sh["pts"]
    rel = sh["rel"]

    lb = np.full(NPTS, -1.0, F32)       # bin - window base (-1 for dummies)
    wbase = np.zeros(NCHUNK, np.int32)  # window base per 512-pt chunk
    for c in range(NCHUNK):
        s, e = c * 512, min((c + 1) * 512, n)
        if s >= n:
            break
        base = int(rel[s])
        span = int(rel[e - 1]) - base + 1
        assert span <= WIN, f"window overflow: span={span}"
        wbase[c] = base
        lb[s:e] = (rel[s:e] - base).astype(F32)

    cnt = np.bincount(rel, minlength=NBINS).astype(F32)
    rcnt = (F32(1.0) / np.maximum(cnt, F32(1.0)))
    rpb = np.zeros((128, NRANK), F32)   # bin-major wrap of rcnt (means + head)
    rpb[:, :] = rcnt[:NROWS].reshape(NRANK, 128).T

    gbin = np.full(NPTS, NROWS - 1, np.int64)          # dummies -> last row
    gbin[:n] = (rel[:n] % 128) * NRANK + rel[:n] // 128
    gidx16 = _wrap_rep(gbin.astype(np.int16))                     # [128, NPTS//16]

    # layouts the device wants
    pts4 = np.zeros((4, NPTS), F32)
    pts4[:3] = pts.T
    pts_flat = np.ascontiguousarray(pts4).reshape(128, FW)
    lbT = np.ascontiguousarray(lb.reshape(NTILES, 128).T)         # [128, NTILES]
    wb = np.zeros((1, 64), np.int32)
    wb[0, :NCHUNK] = wbase
    return dict(pts_flat=pts_flat, lbT=lbT, wbase=wb, rpb=rpb,
                gidx=gidx16)


def weight_inputs(W_pos, b_pos, W0, b0, W1, b1, Ws, Wc, b_c):
    import ml_dtypes
    bf = ml_dtypes.bfloat16
    W_pos, W0, W1, Ws, Wc = [np.ascontiguousarray(x, F32)
                             for x in (W_pos, W0, W1, Ws, Wc)]
    wpos4 = np.zeros((4, D2), F32)
    wpos4[:3] = W_pos
    wposH = _bf16(wpos4)
    wposL = _bf16(wpos4 - wposH)

    w0a = np.ascontiguousarray(W0[:, :HID, :])      # [5,128,128] f32
    w0b = np.ascontiguousarray(W0[:, HID:, :])
    wsa = np.ascontiguousarray(Ws[:, :HID, :])
    wsb = np.ascontiguousarray(Ws[:, HID:, :])
    wsaH = _bf16(wsa)
    wsaL = _bf16(wsa - wsaH)
    wsbH = wsb.astype(np.float16).astype(F32)       # f16 pair (matches pooled)
    wsbL = (wsb - wsbH).astype(np.float16).astype(F32)
    wcH = _bf16(Wc)
    wcL = _bf16(Wc - wcH)

    bias = np.zeros((128, NBIAS), F32)
    bias[:, 0] = np.asarray(b_pos, F32)[:128]
    bias[:, 1] = np.asarray(b_pos, F32)[128:]
    bias[:, 2:7] = np.asarray(b0, F32).T
    bias[:, 7:12] = np.asarray(b1, F32).T
    bias[:, 12] = np.asarray(b_c, F32)
    iota_bc = np.broadcast_to(np.arange(WIN, dtype=F32), (128, WIN)).astype(np.float16)
    ident = np.eye(128, dtype=F32)
    return dict(wposH=wposH.astype(bf), wposL=wposL.astype(bf),
                w0a=w0a, w0b=w0b, w1=W1, wsb0=np.ascontiguousarray(wsb[0]),
                wsa0=np.ascontiguousarray(wsa[0]),
                wsaH=wsaH.astype(bf), wsaL=wsaL.astype(bf),
                wsbH=wsbH.astype(np.float16), wsbL=wsbL.astype(np.float16),
                wcH=wcH.astype(bf), wcL=wcL.astype(bf),
                bias=bias, iota_bc=iota_bc, ident=ident)


# ================================================================ bass build
def build_bass():
    if "/opt/trn_rl_repo" not in sys.path:
        sys.path.insert(0, "/opt/trn_rl_repo")
    import concourse.bass as bass
    import concourse.mybir as mybir
    from concourse import bacc, tile, library_config  # noqa: F401
    from contextlib import ExitStack

    dt = mybir.dt.float32
    bf16 = mybir.dt.bfloat16
    f16 = mybir.dt.float16
    i16 = mybir.dt.int16
    AF = mybir.ActivationFunctionType
    OP = mybir.AluOpType
    GELU = AF.Gelu_apprx_tanh
    EV = mybir.EngineType

    nc = bacc.Bacc("TRN2", num_swdge_queues=4)
    # -------- dram io
    d_pts = nc.dram_tensor("pts_flat", [128, FW], dt, kind="ExternalInput")
    d_lbT = nc.dram_tensor("lbT", [128, NTILES], dt, kind="ExternalInput")
    d_wb = nc.dram_tensor("wbase", [1, 64], mybir.dt.int32, kind="ExternalInput")
    d_rpb = nc.dram_tensor("rpb", [128, NRANK], dt, kind="ExternalInput")
    d_gidx = nc.dram_tensor("gidx", [128, NPTS // 16], i16, kind="ExternalInput")
    d_wposH = nc.dram_tensor("wposH", [4, D2], bf16, kind="ExternalInput")
    d_wposL = nc.dram_tensor("wposL", [4, D2], bf16, kind="ExternalInput")
    d_w0a = nc.dram_tensor("w0a", [NBLK, HID, HID], dt, kind="ExternalInput")
    d_w0b = nc.dram_tensor("w0b", [NBLK, HID, HID], dt, kind="ExternalInput")
    d_w1 = nc.dram_tensor("w1", [NBLK, HID, HID], dt, kind="ExternalInput")
    d_wsb0 = nc.dram_tensor("wsb0", [HID, HID], dt, kind="ExternalInput")
    d_wsa0 = nc.dram_tensor("wsa0", [HID, HID], dt, kind="ExternalInput")
    d_wsaH = nc.dram_tensor("wsaH", [NBLK, HID, HID], bf16, kind="ExternalInput")
    d_wsaL = nc.dram_tensor("wsaL", [NBLK, HID, HID], bf16, kind="ExternalInput")
    d_wsbH = nc.dram_tensor("wsbH", [NBLK, HID, HID], f16, kind="ExternalInput")
    d_wsbL = nc.dram_tensor("wsbL", [NBLK, HID, HID], f16, kind="ExternalInput")
    d_wcH = nc.dram_tensor("wcH", [HID, HID], bf16, kind="ExternalInput")
    d_wcL = nc.dram_tensor("wcL", [HID, HID], bf16, kind="ExternalInput")
    d_bias = nc.dram_tensor("bias", [128, NBIAS], dt, kind="ExternalInput")
    d_iotab = nc.dram_tensor("iota_bc", [128, WIN], f16, kind="ExternalInput")
    d_ident = nc.dram_tensor("ident", [128, 128], dt, kind="ExternalInput")
    d_out = nc.dram_tensor("out_grid", [NROWS, HID], dt, kind="ExternalOutput")
    d_scrH = nc.dram_tensor("pt_scratchH", [4, NPTS], bf16)
    d_scrL = nc.dram_tensor("pt_scratchL", [4, NPTS], bf16)
    # ping-pong DRAM mean tables, row b = [hi(128 f16) | lo(128 f16)]
    d_means = [nc.dram_tensor(f"means{i}", [NROWS, 256], f16)
               for i in range(2)]

    with tile.TileContext(nc) as tc, ExitStack() as ctx:
        cpool = ctx.enter_context(tc.tile_pool(name="const", bufs=1))
        spool = ctx.enter_context(tc.tile_pool(name="stage", bufs=3))
        wpool = ctx.enter_context(tc.tile_pool(name="wts", bufs=2))
        psA = ctx.enter_context(tc.tile_pool(name="psA", bufs=4, space="PSUM"))
        psB = ctx.enter_context(tc.tile_pool(name="psB", bufs=2, space="PSUM"))
        psC = ctx.enter_context(tc.tile_pool(name="psC", bufs=2, space="PSUM"))
        gpool16 = ctx.enter_context(tc.tile_pool(name="gp16", bufs=4))

        breg = {ev: nc.alloc_registers(f"wbase_{ev.name}", engines=(ev,))
                for ev in (EV.DVE,)}

        def load_base(c, ev):
            nc.engines[ev].reg_load(breg[ev], wb[0:1, c:c + 1])
            return nc.snap(breg[ev], donate=True, min_val=0,
                           max_val=NBINS - WIN)

        # ---------------- persistent sbuf
        net = cpool.tile([128, NPTS], dt, tag="net")
        sums = cpool.tile([128, NBINS], dt, tag="sums")
        sumsT16 = cpool.tile([128, NRANK * 256], f16, tag="sumsT16")
        lbT = cpool.tile([128, NTILES], dt, tag="lbT")
        wb = cpool.tile([1, 64], mybir.dt.int32, tag="wb")
        rpb = cpool.tile([128, NRANK], dt, tag="rpb")
        gidx = cpool.tile([128, NPTS // 16], i16, tag="gidx")
        bias = cpool.tile([128, NBIAS], dt, tag="bias")
        iotab = cpool.tile([128, WIN], f16, tag="iotab")
        ident = cpool.tile([128, 128], dt, tag="ident")
        wposH = cpool.tile([4, D2], bf16, tag="wposH")
        wposL = cpool.tile([4, D2], bf16, tag="wposL")
        wsa0 = cpool.tile([128, HID], dt, tag="wsa0")
        wsb0 = cpool.tile([128, HID], dt, tag="wsb0")
        wcH = cpool.tile([128, HID], bf16, tag="wcH")
        wcL = cpool.tile([128, HID], bf16, tag="wcL")

        nc.sync.dma_start(lbT[:], d_lbT[:])
        nc.sync.dma_start(wb[:], d_wb[:])
        nc.sync.dma_start(rpb[:], d_rpb[:])
        nc.sync.dma_start(gidx[:], d_gidx[:])
        nc.sync.dma_start(bias[:], d_bias[:])
        nc.sync.dma_start(iotab[:], d_iotab[:])
        nc.sync.dma_start(ident[:], d_ident[:])
        nc.sync.dma_start(wposH[:], d_wposH[:])
        nc.sync.dma_start(wposL[:], d_wposL[:])
        nc.sync.dma_start(wsa0[:], d_wsa0[:])
        nc.sync.dma_start(wsb0[:], d_wsb0[:])
        nc.sync.dma_start(wcH[:], d_wcH[:])
        nc.sync.dma_start(wcL[:], d_wcL[:])

        def block_weights(i, need_wsa):
            """Stream block-i weights into single-buffered slots."""
            w = {}
            for nm, src in (("w0a", d_w0a), ("w0b", d_w0b), ("w1", d_w1)):
                t = wpool.tile([128, HID], dt, tag=nm)
                nc.sync.dma_start(t[:], src[i, :, :])
                w[nm] = t
            if need_wsa:
                for nm, src, dt_ in (("wsaH", d_wsaH, bf16), ("wsaL", d_wsaL, bf16),
                                     ("wsbH", d_wsbH, f16), ("wsbL", d_wsbL, f16)):
                    t = wpool.tile([128, HID], dt_, tag=nm)
                    nc.sync.dma_start(t[:], src[i, :, :])
                    w[nm] = t
            return w

        def hi_view(ap):
            """bf16 hi part of an f32 AP as a free stride-2 bitcast view."""
            return ap.bitcast(bf16)[:, 1::2]

        # ---------------- pt = 2*frac(clip(p+.5)*res) - 1, flat layout (halves)
        HF = FW // 2
        scrH_flat = d_scrH[:].rearrange("a (b f) -> (a b) f", f=FW)
        scrL_flat = d_scrL[:].rearrange("a (b f) -> (a b) f", f=FW)
        for h in range(2):
            hs = slice(h * HF, (h + 1) * HF)
            pflat = spool.tile([128, HF], dt, tag="pbg", name="pflat")
            nc.sync.dma_start(pflat[:], d_pts[:, hs])
            nc.vector.tensor_scalar(pflat[:], pflat[:], 0.5, 1.0 - 1e-6, OP.add, OP.min)
            nc.vector.tensor_scalar(pflat[:], pflat[:], 1e-6, float(RES), OP.max, OP.mult)
            ci = spool.tile([128, HF], mybir.dt.int32, tag="S2", name="ci")
            nc.vector.tensor_copy(ci[:], pflat[:])
            cf = spool.tile([128, HF], dt, tag="S3", name="cf")
            nc.vector.tensor_copy(cf[:], ci[:])
            nc.vector.tensor_tensor(pflat[:], pflat[:], cf[:], OP.subtract)
            m1 = spool.tile([128, HF], dt, tag="S4", name="m1")
            nc.vector.tensor_scalar(m1[:], pflat[:], 0.0, None, OP.is_lt)
            nc.vector.tensor_tensor(pflat[:], pflat[:], m1[:], OP.add)
            nc.vector.tensor_scalar(pflat[:], pflat[:], 2.0, -1.0, OP.mult, OP.add)
            ptH = spool.tile([128, HF], bf16, tag="OH", name="ptH")
            nc.vector.tensor_copy(ptH[:], pflat[:])
            ptL = spool.tile([128, HF], bf16, tag="S2", name="ptL")
            nc.vector.tensor_tensor(ptL[:], pflat[:], ptH[:], OP.subtract)
            nc.sync.dma_start(scrH_flat[:, hs], ptH[:])
            nc.sync.dma_start(scrL_flat[:, hs], ptL[:])

        # ---------------- setup: pos-mlp + resblock 0, per 512-chunk
        w0 = block_weights(0, need_wsa=False)
        for c in range(NCHUNK):
            cs = slice(c * 512, (c + 1) * 512)
            ptc = spool.tile([4, 1024], bf16, tag="hilo", name="ptc")
            ptcH = ptc[:, 0:512]
            ptcL = ptc[:, 512:1024]
            nc.sync.dma_start(ptcH, d_scrH[:, cs])
            nc.sync.dma_start(ptcL, d_scrL[:, cs])
            x0a = psA.tile([128, 512], dt, tag="pA", name="x0a")
            x0b = psB.tile([128, 512], dt, tag="pB", name="x0b")
            nc.tensor.matmul(x0a[:], wposH[:, 0:128], ptcH, start=True, stop=False)
            nc.tensor.matmul(x0a[:], wposH[:, 0:128], ptcL, start=False, stop=False)
            nc.tensor.matmul(x0a[:], wposL[:, 0:128], ptcH, start=False, stop=True)
            nc.tensor.matmul(x0b[:], wposH[:, 128:256], ptcH, start=True, stop=False)
            nc.tensor.matmul(x0b[:], wposH[:, 128:256], ptcL, start=False, stop=False)
            nc.tensor.matmul(x0b[:], wposL[:, 128:256], ptcH, start=False, stop=True)
            gxa = spool.tile([128, 512], dt, tag="S2", name="gxa")
            gxb = spool.tile([128, 512], dt, tag="S3", name="gxb")
            rxa = spool.tile([128, 512], dt, tag="S4", name="rxa")
            rxb = spool.tile([128, 512], dt, tag="OH", name="rxb")
            nc.scalar.activation(gxa[:], x0a[:], GELU, bias=bias[:, 0:1])
            nc.scalar.activation(gxb[:], x0b[:], GELU, bias=bias[:, 1:2])
            nc.vector.tensor_scalar(rxa[:], x0a[:], bias[:, 0:1], None, OP.add)
            nc.vector.tensor_scalar(rxb[:], x0b[:], bias[:, 1:2], None, OP.add)
            hp = psB.tile([128, 512], dt, tag="pB", name="hp0")
            nc.tensor.matmul(hp[:], w0["w0a"][:], gxa[:], start=True, stop=False)
            nc.tensor.matmul(hp[:], w0["w0b"][:], gxb[:], start=False, stop=True)
            gh = spool.tile([128, 512], dt, tag="S2", name="gh0")
            nc.scalar.activation(gh[:], hp[:], GELU, bias=bias[:, 2:3])
            npp = psA.tile([128, 512], dt, tag="pA", name="npp0")
            nc.tensor.matmul(npp[:], w0["w1"][:], gh[:], start=True, stop=False)
            nc.tensor.matmul(npp[:], wsa0[:], rxa[:], start=False, stop=False)
            nc.tensor.matmul(npp[:], wsb0[:], rxb[:], start=False, stop=True)
            nc.vector.tensor_scalar(net[:, cs], npp[:], bias[:, 7:8], None, OP.add)

        # ---------------- scatter: bf16 hi/lo one-hot matmuls into windows
        def scatter_pass(src_of_chunk):
            nc.vector.memset(sums[:], 0.0)
            for c in range(NCHUNK):
                src = src_of_chunk(c)
                tp = psB.tile([128, 512], dt, tag="pB", name="tp")
                for t in range(4):
                    nc.tensor.transpose(tp[:, t * 128:(t + 1) * 128],
                                        src[:, t * 128:(t + 1) * 128], ident[:])
                hilo = spool.tile([128, 1024], bf16, tag="hilo", name="hilo")
                hiT = hilo[:, 0:512]
                loT = hilo[:, 512:1024]
                nc.scalar.activation(hiT, tp[:], AF.Copy)
                nc.vector.tensor_tensor(loT, tp[:], hiT, OP.subtract)
                sp = psC.tile([128, WIN], dt, tag="pC", name="sp")
                for t in range(4):
                    oh = spool.tile([128, WIN], bf16, tag="OH", name="ohs")
                    col = slice(4 * c + t, 4 * c + t + 1)
                    nc.vector.tensor_scalar(oh[:], iotab[:], lbT[:, col], None,
                                            OP.is_equal)
                    nc.tensor.matmul(sp[:], hilo[:, t * 128:(t + 1) * 128],
                                     oh[:], start=(t == 0), stop=False)
                    nc.tensor.matmul(sp[:], hilo[:, 512 + t * 128:512 + (t + 1) * 128],
                                     oh[:], start=False, stop=(t == 3))
                base = load_base(c, EV.DVE)
                dst = sums[:, bass.ds(base, WIN)]
                nc.vector.tensor_tensor(dst, dst, sp[:], OP.add)

        # ---------------- means: bin-major transpose + 1/cnt + hi/lo f16
        def means_pass(it):
            dmean = d_means[it % 2]
            for j in range(NRANK):
                mtp = psC.tile([128, 128], dt, tag="pC", name="mtp")
                nc.tensor.transpose(mtp[:], sums[:, j * 128:(j + 1) * 128],
                                    ident[:])
                hi = sumsT16[:, j * 256:j * 256 + 128]
                nc.scalar.activation(hi, mtp[:], AF.Identity,
                                     scale=rpb[:, j:j + 1])
                nc.vector.scalar_tensor_tensor(
                    sumsT16[:, j * 256 + 128:(j + 1) * 256], mtp[:],
                    rpb[:, j:j + 1], hi, OP.mult, OP.subtract)
            nc.sync.dma_start(
                dmean[:].rearrange("(p r) c -> p (r c)", p=128), sumsT16[:])
            return dmean

        # ---------------- pooling iterations
        for i in range(1, NBLK):
            wi = block_weights(i, need_wsa=True)
            scatter_pass(lambda c: net[:, c * 512:(c + 1) * 512])
            dmean = means_pass(i)

            for c in range(NCHUNK):
                cs = slice(c * 512, (c + 1) * 512)
                ncur = net[:, cs]
                pooled16 = gpool16.tile([128, 2, 512], f16, tag="pool16")
                nc.gpsimd.dma_gather(
                    pooled16[:], dmean[:],
                    gidx[:, c * 32:(c + 1) * 32],
                    512, 512, 256, transpose=True, queue_num=c % 4)
                pooled = spool.tile([128, 512], dt, tag="S3", name="pooled")
                nc.vector.tensor_tensor(pooled[:], pooled16[:, 0, :],
                                        pooled16[:, 1, :], OP.add)
                gpool = spool.tile([128, 512], dt, tag="S4", name="gpool")
                nc.scalar.activation(gpool[:], pooled[:], GELU)
                gnet = spool.tile([128, 512], dt, tag="S2", name="gnet")
                nc.scalar.activation(gnet[:], ncur, GELU)
                hp = psB.tile([128, 512], dt, tag="pB", name="hpi")
                nc.tensor.matmul(hp[:], wi["w0a"][:], gnet[:], start=True, stop=False)
                nc.tensor.matmul(hp[:], wi["w0b"][:], gpool[:], start=False, stop=True)
                gh = spool.tile([128, 512], dt, tag="hilo", name="ghi")
                nc.scalar.activation(gh[:], hp[:], GELU, bias=bias[:, 2 + i:3 + i])
                netL = spool.tile([128, 512], bf16, tag="netL", name="netL")
                nc.vector.tensor_tensor(netL[:], ncur, hi_view(ncur), OP.subtract)
                npp = psA.tile([128, 512], dt, tag="pA", name="nppi")
                nc.tensor.matmul(npp[:], wi["w1"][:], gh[:], start=True, stop=False)
                nc.tensor.matmul(npp[:], wi["wsaH"][:], hi_view(ncur), start=False, stop=False)
                nc.tensor.matmul(npp[:], wi["wsaH"][:], netL[:], start=False, stop=False)
                nc.tensor.matmul(npp[:], wi["wsaL"][:], hi_view(ncur), start=False, stop=False)
                nc.tensor.matmul(npp[:], wi["wsbH"][:], pooled16[:, 0, :], start=False, stop=False)
                nc.tensor.matmul(npp[:], wi["wsbH"][:], pooled16[:, 1, :], start=False, stop=False)
                nc.tensor.matmul(npp[:], wi["wsbL"][:], pooled16[:, 0, :], start=False, stop=True)
                nc.vector.tensor_scalar(ncur, npp[:], bias[:, 7 + i:8 + i], None, OP.add)

        # ---------------- head: c = net @ Wc + b_c, scatter, scaled out
        def head_chunk(c):
            cs = slice(c * 512, (c + 1) * 512)
            ncur = net[:, cs]
            netL = spool.tile([128, 512], bf16, tag="netL", name="netLh")
            nc.vector.tensor_tensor(netL[:], ncur, hi_view(ncur), OP.subtract)
            cp = psB.tile([128, 512], dt, tag="pB", name="cp")
            nc.tensor.matmul(cp[:], wcH[:], hi_view(ncur), start=True, stop=False)
            nc.tensor.matmul(cp[:], wcH[:], netL[:], start=False, stop=False)
            nc.tensor.matmul(cp[:], wcL[:], hi_view(ncur), start=False, stop=True)
            cv = spool.tile([128, 512], dt, tag="S2", name="cv")
            nc.scalar.activation(cv[:], cp[:], AF.Identity, bias=bias[:, 12:13])
            return cv[:]

        scatter_pass(head_chunk)
        for j in range(NRANK):
            mtp = psC.tile([128, 128], dt, tag="pC", name="mtph")
            nc.tensor.transpose(mtp[:], sums[:, j * 128:(j + 1) * 128], ident[:])
            ostg = spool.tile([128, 128], dt, tag="S3", name="ostg")
            nc.scalar.activation(ostg[:], mtp[:], AF.Identity,
                                 scale=rpb[:, j:j + 1])
            nc.sync.dma_start(d_out[j * 128:(j + 1) * 128, :], ostg[:])

    return nc


# ================================================================ run + glue
_BUILT = {}


def get_nc():
    if "nc" not in _BUILT:
        nc = build_bass()
        nc.compile()
        _BUILT["nc"] = nc
    return _BUILT["nc"]


def make_in_maps(p, sparse_coords, W_pos, b_pos, W0, b0, W1, b1, Ws, Wc, b_c, res):
    index, counts = point_meta(p, sparse_coords, int(res))
    shards = shard(np.asarray(p, F32), index)
    wdict = weight_inputs(W_pos, b_pos, W0, b0, W1, b1, Ws, Wc, b_c)
    in_maps = []
    for sh in shards:
        ci = core_inputs(sh)
        m = dict(ci)
        m.update(wdict)
        in_maps.append(m)
    return in_maps, shards, counts


def assemble(results, shards, counts, sparse_coords):
    sc = np.asarray(sparse_coords)
    starts = np.concatenate([[0], np.cumsum(counts)[:-1]])
    out = np.zeros((sc.shape[0], HID), F32)
    for sh, r_ in zip(shards, results):
        tab = np.asarray(r_["out_grid"])              # [NBINS, 128]
        lo, hi, b = sh["lo"], sh["hi"], sh["batch"]
        hi_eff = min(hi, int(counts[b]))
        if hi_eff > lo:
            out[starts[b] + lo: starts[b] + hi_eff] = tab[0:hi_eff - lo, :]
    return out


def kernel(p, sparse_coords, W_pos, b_pos, W0, b0, W1, b1, Ws, Wc, b_c, res):
    if "/opt/trn_rl_repo" not in sys.path:
        sys.path.insert(0, "/opt/trn_rl_repo")
    from concourse.bass_utils import run_bass_kernel_spmd

    in_maps, shards, counts = make_in_maps(
        p, sparse_coords, W_pos, b_pos, W0, b0, W1, b1, Ws, Wc, b_c, res)
    nc = get_nc()
    results = run_bass_kernel_spmd(nc, in_maps, list(range(NCORES))).results
    return assemble(results, shards, counts, sparse_coords)
